# revision 6
# baseline (speedup 1.0000x reference)
"""TRN2 Bass kernel for the NTK-track Conv1d problem.

Reference computation (per batch element b, all fp32):
    xv = relu(x[...,0]); x0 = relu(x[...,1]); dx = x[...,2] * (x[...,1] >= 0)
    s = sqrt(|alpha|)  (per-tap scale, K=9)
    x_out  = conv1d(xv, weight*s)/sqrt(C) + bias*sqrt(|beta|)
    x0_out = conv1d(x0, w0*s)/sqrt(C)     + b0*sqrt(|beta|)
    dx_out = (conv1d(dx, w0*s) + conv1d(x0, w*s))/sqrt(C) + b*sqrt(|beta|)
    out = stack([x_out, x0_out, dx_out], -1)

Shapes: x (8, 256, 8192, 3); weight/w0/w (256, 256, 9); pad=4 (same conv).

Strategy: data-parallel over batch (8 cores, 1 batch element each).
Per core the four conv streams (xv*W1, x0*W2, dx*W2, x0*W3) are shifted
128x128 matmuls accumulated in PSUM (contraction over C chunks and taps).
Mixed precision per tap:
  - taps NOT in FP8_TAPS run in bf16 (1 col/cycle, ~2e-3 global err),
  - taps in FP8_TAPS run as e4m3 DoubleRow matmuls (2 cols/cycle): the
    stationary operand packs host-prepared (w_hi, w_lo) e4m3 pairs so the
    weight is ~7-mantissa-bit accurate, the moving operand is a single
    e4m3 copy of the track broadcast across the pair dim; the only extra
    error is the track quantization on those taps (~6e-3/tap global).
The 1/sqrt(C) and sqrt(|alpha|) factors are folded into the weights on the
host; the sqrt(|beta|)-scaled biases are added during PSUM->SBUF eviction.
Input x is pre-converted to bf16 on the host (halves the slab DMA).

DMA order is tuned so compute starts early: the first time-tile's input
slabs are fetched first, then weight chunks in exact consumption order.
"""

import math

import numpy as np

B, C, O, T, K = 8, 256, 256, 8192, 9
PAD = 4
P = 128  # partitions
TT = 512  # time-tile (matmul free dim)
NT = T // TT  # 16 time tiles
CCH = C // P  # 2 contraction chunks
OCH = O // P  # 2 output-partition chunks
HALO = TT + 2 * PAD  # 520 input columns per tile
NCORES = 8
FP8_TAPS = (0, 8)  # taps computed in e4m3 DoubleRow (2x PE rate)


def _split_excess_waits(nc) -> int:
    """Move excess per-instruction semaphore waits onto standalone
    EventSemaphore carrier instructions.

    The walrus build in this environment rejects any instruction carrying
    more than ONE sync wait at codegen ("Too many sync wait commands");
    Tile's sem assignment freely emits several. Walk the finished BIR and
    hoist overflow waits onto fresh same-engine EventSemaphore instructions
    placed immediately before the over-budget instruction.
    """
    import concourse.mybir as mybir

    n_carriers = 0
    for f in nc.m.functions:
        for blk in f.blocks:
            insts = list(blk.instructions)
            new_insts = []
            dirty = False
            for inst in insts:
                si = inst.sync_info
                waits = list(si.on_wait) if si is not None and si.on_wait else []
                if len(waits) > 1:
                    overflow, keep = waits[:-1], waits[-1:]
                    for w in overflow:
                        ev = mybir.InstEventSemaphore(
                            name=f"{inst.name}_waitc{n_carriers}",
                            engine=inst.engine,
                        )
                        ev.sync_info = mybir.SyncInfo(on_wait=[w], on_update=[])
                        nc.register_instruction(ev, overwrite=True)
                        new_insts.append(ev)
                        n_carriers += 1
                    upd = list(si.on_update) if si.on_update else []
                    inst.sync_info = mybir.SyncInfo(on_wait=keep, on_update=upd)
                    dirty = True
                new_insts.append(inst)
            if dirty:
                blk.instructions = new_insts
    return n_carriers


def _dedupe_ldweights(nc) -> int:
    """Drop an InstLdweights whose weights AP matches the previous kept
    InstLdweights on the same stream with only Matmult / EventSemaphore
    instructions in between (the PE array still holds those weights).
    Waits from a dropped LDW migrate to the next kept PE instruction.
    Must run BEFORE _split_excess_waits so merged waits get re-split."""
    import concourse.mybir as mybir

    removed = 0
    for f in nc.m.functions:
        for blk in f.blocks:
            insts = list(blk.instructions)
            new_insts = []
            last_ld_key = None
            pend_waits = []
            for inst in insts:
                op = inst.opcode
                if op == "Ldweights":
                    key = str(inst.ins[0])
                    if key == last_ld_key:
                        si = inst.sync_info
                        if si is not None and si.on_wait:
                            pend_waits.extend(list(si.on_wait))
                        if si is not None and si.on_update:
                            # don't drop an LDW other procs wait on
                            new_insts.append(inst)
                            continue
                        removed += 1
                        continue
                    last_ld_key = key
                elif op in ("Matmult", "EventSemaphore"):
                    pass  # doesn't clobber the loaded weights
                else:
                    last_ld_key = None
                if pend_waits and inst.engine == mybir.EngineType.PE:
                    si = inst.sync_info
                    w = list(si.on_wait) if si is not None and si.on_wait else []
                    u = list(si.on_update) if si is not None and si.on_update else []
                    inst.sync_info = mybir.SyncInfo(on_wait=pend_waits + w, on_update=u)
                    pend_waits = []
                new_insts.append(inst)
            if removed:
                assert not pend_waits
                blk.instructions = new_insts
    return removed


def _build_nc(fp8_taps: tuple = FP8_TAPS):
    import concourse.bass as bass
    import concourse.mybir as mybir
    from concourse.tile import TileContext

    f32 = mybir.dt.float32
    bf16 = mybir.dt.bfloat16
    f8 = mybir.dt.float8e4
    AF = mybir.ActivationFunctionType
    OP = mybir.AluOpType
    PM = mybir.MatmulPerfMode.DoubleRow

    fp8_taps = tuple(fp8_taps)
    bf_taps = tuple(k for k in range(K) if k not in fp8_taps)
    NB = len(bf_taps)
    NQ = len(fp8_taps)
    NH = TT // 256  # DoubleRow half-tiles per time tile

    nc = bass.Bass()
    xd = nc.declare_dram_parameter("xd", [C, T * 3], bf16, isOutput=False)
    # bf16 weights: [c_lo, tensor, cc, kb, oc, o_lo] flattened after c_lo
    wb = nc.declare_dram_parameter("wb", [P, 3 * CCH * NB * OCH * P], bf16,
                                   isOutput=False)
    # fp8 (hi, lo) pairs: [c_lo, tensor, cc, kq, oc, pair, o_lo]
    if NQ:
        wq = nc.declare_dram_parameter("wq", [P, 3 * CCH * NQ * OCH * 2 * P], f8,
                                       isOutput=False)
    bs = nc.declare_dram_parameter("bs", [P, OCH * 3], f32, isOutput=False)
    yd = nc.declare_dram_parameter("yd", [C, T * 3], f32, isOutput=True)

    with TileContext(nc) as tc:
        with (
            tc.tile_pool(name="wpool", bufs=1) as wpool,
            tc.tile_pool(name="slabs", bufs=4) as slabs,
            tc.tile_pool(name="trks", bufs=4) as trks,
            tc.tile_pool(name="opool", bufs=4) as opool,
            tc.tile_pool(name="psum", bufs=2, space="PSUM") as psp,
        ):
            # Persistent weights / biases
            wbs = wpool.tile([P, 3, CCH, NB, OCH, P], bf16)
            if NQ:
                wqs = wpool.tile([P, 3, CCH, NQ, OCH, 2, P], f8)
            bss = wpool.tile([P, OCH, 3], f32)

            def dma_slabs(tt, interleave_dma=None):
                t0 = tt * TT
                out = []
                for cc in range(CCH):
                    if interleave_dma is not None and cc:
                        # keep the DMA queue in consumption order: the first
                        # weight chunk lands right after the first slab
                        interleave_dma(0, cc - 1)
                    slab = slabs.tile([P, HALO * 3], bf16, tag="slab")
                    lo = 3 * (t0 - PAD)
                    hi = 3 * (t0 + TT + PAD)
                    zlo = max(0, -lo)      # zero-pad columns at the left edge
                    zhi = max(0, hi - 3 * T)  # and at the right edge
                    if zlo:
                        nc.vector.memset(slab[:, :zlo], 0.0)
                    if zhi:
                        nc.vector.memset(slab[:, HALO * 3 - zhi:], 0.0)
                    nc.sync.dma_start(
                        slab[:, zlo : HALO * 3 - zhi],
                        xd[cc * P : (cc + 1) * P, lo + zlo : hi - zhi],
                    )
                    out.append(slab)
                return out

            def prep_tracks(slab_pair):
                tracks = []
                for cc in range(CCH):
                    slab = slab_pair[cc]
                    sv = slab[:].rearrange("p (t s) -> p t s", s=3)
                    trk = trks.tile([P, 3, HALO], bf16, tag="trk")
                    # xv = relu(track0); x0 = relu(track1)  (ACT engine)
                    nc.scalar.activation(trk[:, 0], sv[:, :, 0], AF.Relu)
                    nc.scalar.activation(trk[:, 1], sv[:, :, 1], AF.Relu)
                    # dx = track2 * (track1 >= 0)  (DVE engine)
                    msk = trks.tile([P, HALO], bf16, tag="msk")
                    nc.vector.tensor_scalar(msk[:], sv[:, :, 1], 0.0, None, OP.is_ge)
                    nc.vector.tensor_tensor(trk[:, 2], msk[:], sv[:, :, 2], OP.mult)
                    if NQ:
                        x8 = trks.tile([P, 3, HALO], f8, tag="x8")
                        # fp8 copies of the tracks for the DoubleRow taps
                        nc.scalar.activation(x8[:, 0], sv[:, :, 0], AF.Relu)
                        nc.scalar.activation(x8[:, 1], sv[:, :, 1], AF.Relu)
                        nc.vector.tensor_copy(x8[:, 2], trk[:, 2])
                        tracks.append((trk, x8))
                    else:
                        tracks.append((trk, None))
                return tracks

            def post(oc, t0, ps_x, ps_x0, ps_dx, c0=0, c1=TT):
                w = c1 - c0
                ot = opool.tile([P, w, 3], f32, tag=f"ot{w}")
                nc.vector.tensor_scalar_add(ot[:, :, 0], ps_x[:, c0:c1], bss[:, oc, 0:1])
                nc.vector.tensor_scalar_add(ot[:, :, 1], ps_x0[:, c0:c1], bss[:, oc, 1:2])
                nc.vector.tensor_scalar_add(ot[:, :, 2], ps_dx[:, c0:c1], bss[:, oc, 2:3])
                nc.sync.dma_start(
                    yd[oc * P : (oc + 1) * P, 3 * (t0 + c0) : 3 * (t0 + c1)],
                    ot[:].rearrange("p t s -> p (t s)"),
                )

            def mm_group(tracks, oc):
                """One accumulation group: all taps of the 4 conv streams for
                one (time-tile, oc). Streams: t0=w1*trk0, t1=w2*trk1,
                t2=w2*trk2 + w3*trk1 (dx)."""
                ps_x = psp.tile([P, TT], f32, tag="psx")
                ps_x0 = psp.tile([P, TT], f32, tag="psx0")
                ps_dx = psp.tile([P, TT], f32, tag="psdx")

                def bf_mms(ps, ti, tr, first, last):
                    # bf16 taps of conv stream (weight tensor ti, track tr)
                    for ci, cc in enumerate(range(CCH)):
                        for ki, kb in enumerate(bf_taps):
                            nc.tensor.matmul(
                                ps[:], wbs[:, ti, cc, ki, oc],
                                tracks[cc][0][:, tr, kb : kb + TT],
                                start=(first and ci == 0 and ki == 0),
                                stop=(last and ci == CCH - 1 and ki == NB - 1),
                            )

                def q_mms(ps, ti, tr, last):
                    # fp8 DoubleRow taps (half tiles of 256 columns)
                    for ci, cc in enumerate(range(CCH)):
                        x8 = tracks[cc][1]
                        for ki, kq in enumerate(fp8_taps):
                            for h in range(NH):
                                rhs = x8[:, tr : tr + 1,
                                         kq + 256 * h : kq + 256 * h + 256]
                                nc.tensor.matmul(
                                    ps[:, 256 * h : 256 * h + 256],
                                    wqs[:, ti, cc, ki, oc],
                                    rhs.broadcast_to([P, 2, 256]),
                                    start=False,
                                    stop=(last and ci == CCH - 1
                                          and ki == NQ - 1),
                                    perf_mode=PM,
                                )

                # x stream: w1 on track0
                bf_mms(ps_x, 0, 0, True, not NQ)
                if NQ:
                    q_mms(ps_x, 0, 0, True)
                # x0 stream (w2*trk1) and dx part 1 (w2*trk2): adjacent
                # matmuls share the w2 stationary tile (ldweights dedupe)
                for cc in range(CCH):
                    for ki, kb in enumerate(bf_taps):
                        nc.tensor.matmul(
                            ps_x0[:], wbs[:, 1, cc, ki, oc],
                            tracks[cc][0][:, 1, kb : kb + TT],
                            start=(cc == 0 and ki == 0),
                            stop=(not NQ and cc == CCH - 1 and ki == NB - 1),
                        )
                        nc.tensor.matmul(
                            ps_dx[:], wbs[:, 1, cc, ki, oc],
                            tracks[cc][0][:, 2, kb : kb + TT],
                            start=(cc == 0 and ki == 0),
                            stop=False,
                        )
                if NQ:
                    q_mms(ps_x0, 1, 1, True)
                    q_mms(ps_dx, 1, 2, False)
                # dx part 2: w3 on track1
                bf_mms(ps_dx, 2, 1, False, not NQ)
                if NQ:
                    q_mms(ps_dx, 2, 1, True)
                return ps_x, ps_x0, ps_dx

            # ---- program start: first slab, first weight chunk, second
            # slab, then the remaining weight chunks in consumption order ----
            def dma_wchunk(ti, cc):
                nc.sync.dma_start(
                    wbs[:, ti, cc],
                    wb[:, (ti * CCH + cc) * NB * OCH * P :
                          (ti * CCH + cc + 1) * NB * OCH * P]
                    .rearrange("p (k o q) -> p k o q", k=NB, o=OCH),
                )
                if NQ:
                    nc.sync.dma_start(
                        wqs[:, ti, cc],
                        wq[:, (ti * CCH + cc) * NQ * OCH * 2 * P :
                              (ti * CCH + cc + 1) * NQ * OCH * 2 * P]
                        .rearrange("p (k o i q) -> p k o i q",
                                   k=NQ, o=OCH, i=2),
                    )

            slab0 = dma_slabs(0, interleave_dma=dma_wchunk)
            for ti in range(3):
                for cc in range(CCH):
                    if (ti, cc) == (0, 0):
                        continue  # already issued between the tt0 slabs
                    dma_wchunk(ti, cc)
            nc.sync.dma_start(bss[:], bs[:].rearrange("p (o s) -> p o s", o=OCH))

            for tt in range(NT):
                slab_pair = slab0 if tt == 0 else dma_slabs(tt)
                tracks = prep_tracks(slab_pair)
                for oc in range(OCH):
                    ps_x, ps_x0, ps_dx = mm_group(tracks, oc)
                    post(oc, tt * TT, ps_x, ps_x0, ps_dx)

    ndedup = _dedupe_ldweights(nc)
    if ndedup:
        import logging
        logging.getLogger(__name__).info("deduped %d ldweights", ndedup)
    _split_excess_waits(nc)
    return nc


_CACHE: dict = {}


def _prep_weights(weight, w0, w, alpha, fp8_taps=FP8_TAPS):
    """(O, C, K) fp32 -> bf16 lhsT chunks + e4m3 (hi, lo) pair chunks."""
    import ml_dtypes

    E4 = ml_dtypes.float8_e4m3
    BF = ml_dtypes.bfloat16
    fp8_taps = tuple(fp8_taps)
    bf_taps = tuple(k for k in range(K) if k not in fp8_taps)
    s = np.sqrt(np.abs(np.asarray(alpha, np.float32)))  # (1,1,K)
    inv_sqrt_c = np.float32(1.0 / math.sqrt(C))
    wb_parts = []
    wq_parts = []
    for wt in (weight, w0, w):
        wt = np.asarray(wt, np.float32) * s * inv_sqrt_c  # (O, C, K)
        # lhsT layout: [c_lo, cc, k, oc, o_lo]
        lt = wt.reshape(OCH, P, CCH, P, K).transpose(3, 2, 4, 0, 1)
        # bf16 taps
        wb_parts.append(np.ascontiguousarray(lt[:, :, bf_taps]).astype(BF))
        if fp8_taps:
            q = lt[:, :, fp8_taps]  # (P, CCH, NQ, OCH, P)
            hi = q.astype(E4).astype(np.float32)
            lo = (q - hi).astype(E4)
            pair = np.stack([hi.astype(E4), lo], axis=-2)  # (..., 2, P)
            wq_parts.append(np.ascontiguousarray(pair))
    wb_np = np.concatenate([p.reshape(P, -1) for p in wb_parts], axis=1)
    wq_np = (np.concatenate([p.reshape(P, -1) for p in wq_parts], axis=1)
             if fp8_taps else None)
    return wb_np, wq_np


def kernel(x, weight, w0, w, alpha, bias, b0, b, beta):
    import ml_dtypes
    from concourse.bass_utils import run_bass_kernel_spmd

    x = np.asarray(x, np.float32)
    wb_np, wq_np = _prep_weights(weight, w0, w, alpha)
    sb = np.float32(math.sqrt(abs(float(np.asarray(beta)))))
    biases = np.stack(
        [np.asarray(bias, np.float32) * sb,
         np.asarray(b0, np.float32) * sb,
         np.asarray(b, np.float32) * sb],
        axis=-1,
    )  # (O, 3) in track order [x, x0, dx]
    bs_np = np.ascontiguousarray(biases.reshape(OCH, P, 3).transpose(1, 0, 2)).reshape(
        P, OCH * 3
    )

    if "nc" not in _CACHE:
        _CACHE["nc"] = _build_nc()
    nc = _CACHE["nc"]

    xh = x.reshape(B, C, T * 3).astype(ml_dtypes.bfloat16)
    in_maps = []
    for c in range(NCORES):
        im = {
            "xd": np.ascontiguousarray(xh[c]),
            "wb": wb_np,
            "bs": bs_np,
        }
        if wq_np is not None:
            im["wq"] = wq_np
        in_maps.append(im)
    res = run_bass_kernel_spmd(nc, in_maps, list(range(NCORES)))
    out = np.empty((B, C, T, 3), np.float32)
    for c in range(NCORES):
        out[c] = res.results[c]["yd"].reshape(C, T, 3)
    return out


# revision 20
# speedup vs baseline: 1.9880x; 1.9880x over previous
"""TRN2 Bass kernel for the NTK-track Conv1d problem.

Reference computation (per batch element b, all fp32):
    xv = relu(x[...,0]); x0 = relu(x[...,1]); dx = x[...,2] * (x[...,1] >= 0)
    s = sqrt(|alpha|)  (per-tap scale, K=9)
    x_out  = conv1d(xv, weight*s)/sqrt(C) + bias*sqrt(|beta|)
    x0_out = conv1d(x0, w0*s)/sqrt(C)     + b0*sqrt(|beta|)
    dx_out = (conv1d(dx, w0*s) + conv1d(x0, w*s))/sqrt(C) + b*sqrt(|beta|)
    out = stack([x_out, x0_out, dx_out], -1)

Shapes: x (8, 256, 8192, 3); weight/w0/w (256, 256, 9); pad=4 (same conv).

Strategy: data-parallel over batch (8 cores, 1 batch element each).
Per core the four conv streams (xv*W1, x0*W2, dx*W2, x0*W3) are shifted
128x128 matmuls accumulated in PSUM (contraction over C chunks and taps).
Mixed precision per tap:
  - taps NOT in FP8_TAPS run in bf16 (1 col/cycle, ~2e-3 global err),
  - taps in FP8_TAPS run as e4m3 DoubleRow matmuls (2 cols/cycle): the
    stationary operand packs host-prepared (w_hi, w_lo) e4m3 pairs so the
    weight is ~7-mantissa-bit accurate, the moving operand is a single
    e4m3 copy of the track broadcast across the pair dim; the only extra
    error is the track quantization on those taps (~6e-3/tap global).
The 1/sqrt(C) and sqrt(|alpha|) factors are folded into the weights on the
host; the sqrt(|beta|)-scaled biases are added during PSUM->SBUF eviction.
Input x is pre-converted to bf16 on the host (halves the slab DMA).

DMA order is tuned so compute starts early: the first time-tile's input
slabs are fetched first, then weight chunks in exact consumption order.
"""

import math

import numpy as np

B, C, O, T, K = 8, 256, 256, 8192, 9
PAD = 4
P = 128  # partitions
TT = 512  # time-tile (matmul free dim)
NT = T // TT  # 16 time tiles
CCH = C // P  # 2 contraction chunks
OCH = O // P  # 2 output-partition chunks
HALO = TT + 2 * PAD  # 520 input columns per tile
NCORES = 8
FP8_TAPS = (0, 8)  # taps computed in e4m3 DoubleRow (2x PE rate)


def _split_excess_waits(nc) -> int:
    """Move excess per-instruction semaphore waits onto standalone
    EventSemaphore carrier instructions.

    The walrus build in this environment rejects any instruction carrying
    more than ONE sync wait at codegen ("Too many sync wait commands");
    Tile's sem assignment freely emits several. Walk the finished BIR and
    hoist overflow waits onto fresh same-engine EventSemaphore instructions
    placed immediately before the over-budget instruction.
    """
    import concourse.mybir as mybir

    n_carriers = 0
    for f in nc.m.functions:
        for blk in f.blocks:
            insts = list(blk.instructions)
            new_insts = []
            dirty = False
            for inst in insts:
                si = inst.sync_info
                waits = list(si.on_wait) if si is not None and si.on_wait else []
                if len(waits) > 1:
                    overflow, keep = waits[:-1], waits[-1:]
                    for w in overflow:
                        ev = mybir.InstEventSemaphore(
                            name=f"{inst.name}_waitc{n_carriers}",
                            engine=inst.engine,
                        )
                        ev.sync_info = mybir.SyncInfo(on_wait=[w], on_update=[])
                        nc.register_instruction(ev, overwrite=True)
                        new_insts.append(ev)
                        n_carriers += 1
                    upd = list(si.on_update) if si.on_update else []
                    inst.sync_info = mybir.SyncInfo(on_wait=keep, on_update=upd)
                    dirty = True
                new_insts.append(inst)
            if dirty:
                blk.instructions = new_insts
    return n_carriers


def _dedupe_ldweights(nc) -> int:
    """Drop an InstLdweights whose weights AP matches the previous kept
    InstLdweights on the same stream with only Matmult / EventSemaphore
    instructions in between (the PE array still holds those weights).
    Waits from a dropped LDW migrate to the next kept PE instruction.
    Must run BEFORE _split_excess_waits so merged waits get re-split."""
    import concourse.mybir as mybir

    removed = 0
    for f in nc.m.functions:
        for blk in f.blocks:
            insts = list(blk.instructions)
            new_insts = []
            last_ld_key = None
            pend_waits = []
            for inst in insts:
                op = inst.opcode
                if op == "Ldweights":
                    key = str(inst.ins[0])
                    if key == last_ld_key:
                        si = inst.sync_info
                        if si is not None and si.on_wait:
                            pend_waits.extend(list(si.on_wait))
                        if si is not None and si.on_update:
                            # don't drop an LDW other procs wait on
                            new_insts.append(inst)
                            continue
                        removed += 1
                        continue
                    last_ld_key = key
                elif op in ("Matmult", "EventSemaphore"):
                    pass  # doesn't clobber the loaded weights
                else:
                    last_ld_key = None
                if pend_waits and inst.engine == mybir.EngineType.PE:
                    si = inst.sync_info
                    w = list(si.on_wait) if si is not None and si.on_wait else []
                    u = list(si.on_update) if si is not None and si.on_update else []
                    inst.sync_info = mybir.SyncInfo(on_wait=pend_waits + w, on_update=u)
                    pend_waits = []
                new_insts.append(inst)
            if removed:
                assert not pend_waits
                blk.instructions = new_insts
    return removed


def _build_nc(fp8_taps: tuple = FP8_TAPS, warmup: int = 8):
    import concourse.bass as bass
    import concourse.mybir as mybir
    from concourse.tile import TileContext

    f32 = mybir.dt.float32
    bf16 = mybir.dt.bfloat16
    f8 = mybir.dt.float8e4
    AF = mybir.ActivationFunctionType
    OP = mybir.AluOpType
    PM = mybir.MatmulPerfMode.DoubleRow

    fp8_taps = tuple(fp8_taps)
    bf_taps = tuple(k for k in range(K) if k not in fp8_taps)
    NB = len(bf_taps)
    NQ = len(fp8_taps)
    NH = TT // 256  # DoubleRow half-tiles per time tile

    nc = bass.Bass()
    xd = nc.declare_dram_parameter("xd", [C, T * 3], bf16, isOutput=False)
    # bf16 weights: [c_lo, tensor, cc, kb, oc, o_lo] flattened after c_lo
    wb = nc.declare_dram_parameter("wb", [P, 3 * CCH * NB * OCH * P], bf16,
                                   isOutput=False)
    # fp8 (hi, lo) pairs: [c_lo, tensor, cc, kq, oc, pair, o_lo]
    if NQ:
        wq = nc.declare_dram_parameter("wq", [P, 3 * CCH * NQ * OCH * 2 * P], f8,
                                       isOutput=False)
    bs = nc.declare_dram_parameter("bs", [P, OCH * 3], f32, isOutput=False)
    yd = nc.declare_dram_parameter("yd", [C, T * 3], f32, isOutput=True)

    with TileContext(nc) as tc:
        with (
            tc.tile_pool(name="wpool", bufs=1) as wpool,
            tc.tile_pool(name="slabs", bufs=4) as slabs,
            tc.tile_pool(name="trks", bufs=4) as trks,
            tc.tile_pool(name="opool", bufs=4) as opool,
            tc.tile_pool(name="psum", bufs=2, space="PSUM") as psp,
            tc.tile_pool(name="psumw", bufs=1, space="PSUM") as psw,
        ):
            # Persistent weights / biases
            wbs = wpool.tile([P, 3, CCH, NB, OCH, P], bf16)
            if NQ:
                wqs = wpool.tile([P, 3, CCH, NQ, OCH, 2, P], f8)
            bss = wpool.tile([P, OCH, 3], f32)

            if warmup:
                # Dummy matmuls on zeroed scratch bridge the PE through the
                # initial input/weight DMA wait: the p-state ramp (full
                # speed only after 3us of continuous PE busy) completes
                # before the first real matmul instead of slowing it down.
                scr = wpool.tile([P, TT], bf16)
                psd0 = psw.tile([P, TT], f32)
                nc.vector.memset(scr[:], 0.0)
                for _ in range(warmup):
                    nc.tensor.matmul(psd0[:], scr[:, 0:P], scr[:],
                                     start=True, stop=True)

            def dma_slabs(tt, interleave_dma=None):
                t0 = tt * TT
                out = []
                for cc in range(CCH):
                    if interleave_dma is not None and cc:
                        # keep the DMA queue in consumption order: the first
                        # weight chunk lands right after the first slab
                        interleave_dma(0, cc - 1)
                    slab = slabs.tile([P, HALO * 3], bf16, tag="slab")
                    lo = 3 * (t0 - PAD)
                    hi = 3 * (t0 + TT + PAD)
                    zlo = max(0, -lo)      # zero-pad columns at the left edge
                    zhi = max(0, hi - 3 * T)  # and at the right edge
                    if zlo:
                        nc.vector.memset(slab[:, :zlo], 0.0)
                    if zhi:
                        nc.vector.memset(slab[:, HALO * 3 - zhi:], 0.0)
                    nc.sync.dma_start(
                        slab[:, zlo : HALO * 3 - zhi],
                        xd[cc * P : (cc + 1) * P, lo + zlo : hi - zhi],
                    )
                    out.append(slab)
                return out

            def prep_tracks(slab_pair):
                # phase 1: bf16 tracks for all cc chunks (these gate the
                # first matmuls of the group); phase 2: fp8 copies (used by
                # the DoubleRow passes at the end of each group)
                tracks = []
                for cc in range(CCH):
                    slab = slab_pair[cc]
                    sv = slab[:].rearrange("p (t s) -> p t s", s=3)
                    trk = trks.tile([P, 3, HALO], bf16, tag="trk")
                    # xv = relu(track0); x0 = relu(track1)  (ACT engine)
                    nc.scalar.activation(trk[:, 0], sv[:, :, 0], AF.Relu)
                    nc.scalar.activation(trk[:, 1], sv[:, :, 1], AF.Relu)
                    # dx = track2 * (track1 >= 0)  (DVE engine)
                    msk = trks.tile([P, HALO], bf16, tag="msk")
                    nc.vector.tensor_scalar(msk[:], sv[:, :, 1], 0.0, None, OP.is_ge)
                    nc.vector.tensor_tensor(trk[:, 2], msk[:], sv[:, :, 2], OP.mult)
                    tracks.append((trk, None))
                if NQ:
                    for cc in range(CCH):
                        slab = slab_pair[cc]
                        sv = slab[:].rearrange("p (t s) -> p t s", s=3)
                        trk = tracks[cc][0]
                        x8 = trks.tile([P, 3, HALO], f8, tag="x8")
                        # fp8 copies of the tracks for the DoubleRow taps
                        nc.scalar.activation(x8[:, 0], sv[:, :, 0], AF.Relu)
                        nc.scalar.activation(x8[:, 1], sv[:, :, 1], AF.Relu)
                        nc.vector.tensor_copy(x8[:, 2], trk[:, 2])
                        tracks[cc] = (trk, x8)
                return tracks

            def post(oc, t0, ps_x, ps_x0, ps_dx, c0=0, c1=TT):
                w = c1 - c0
                ot = opool.tile([P, w, 3], f32, tag=f"ot{w}")
                # x-track eviction on ACT (bias via activation), rest on DVE
                nc.scalar.activation(ot[:, :, 0], ps_x[:, c0:c1], AF.Identity,
                                     bias=bss[:, oc, 0:1])
                nc.vector.tensor_scalar_add(ot[:, :, 1], ps_x0[:, c0:c1], bss[:, oc, 1:2])
                nc.vector.tensor_scalar_add(ot[:, :, 2], ps_dx[:, c0:c1], bss[:, oc, 2:3])
                nc.sync.dma_start(
                    yd[oc * P : (oc + 1) * P, 3 * (t0 + c0) : 3 * (t0 + c1)],
                    ot[:].rearrange("p t s -> p (t s)"),
                )

            def mm_group(tracks, oc, c0=0, c1=TT):
                """One accumulation group: all taps of the 4 conv streams for
                one (time-tile, oc), over output columns [c0, c1). Streams:
                t0=w1*trk0, t1=w2*trk1, t2=w2*trk2 + w3*trk1 (dx)."""
                w = c1 - c0
                ps_x = psp.tile([P, TT], f32, tag="psx")
                ps_x0 = psp.tile([P, TT], f32, tag="psx0")
                ps_dx = psp.tile([P, TT], f32, tag="psdx")

                def bf_mms(ps, ti, tr, first, last):
                    # bf16 taps of conv stream (weight tensor ti, track tr)
                    for ci, cc in enumerate(range(CCH)):
                        for ki, kb in enumerate(bf_taps):
                            nc.tensor.matmul(
                                ps[:, c0:c1], wbs[:, ti, cc, ki, oc],
                                tracks[cc][0][:, tr, kb + c0 : kb + c1],
                                start=(first and ci == 0 and ki == 0),
                                stop=(last and ci == CCH - 1 and ki == NB - 1),
                            )

                def q_mms(ps, ti, tr, last):
                    # fp8 DoubleRow taps (half tiles of <=256 columns)
                    for ci, cc in enumerate(range(CCH)):
                        x8 = tracks[cc][1]
                        for ki, kq in enumerate(fp8_taps):
                            for h0 in range(c0, c1, 256):
                                hw = min(256, c1 - h0)
                                rhs = x8[:, tr : tr + 1, kq + h0 : kq + h0 + hw]
                                nc.tensor.matmul(
                                    ps[:, h0 : h0 + hw],
                                    wqs[:, ti, cc, ki, oc],
                                    rhs.broadcast_to([P, 2, hw]),
                                    start=False,
                                    stop=(last and ci == CCH - 1
                                          and ki == NQ - 1),
                                    perf_mode=PM,
                                )

                # bf16 passes first (their tracks/weights are ready
                # earliest at kernel start), fp8 DoubleRow passes last.
                # x stream: w1 on track0
                bf_mms(ps_x, 0, 0, True, not NQ)
                # x0 stream (w2*trk1) and dx part 1 (w2*trk2): adjacent
                # matmuls share the w2 stationary tile (ldweights dedupe)
                for cc in range(CCH):
                    for ki, kb in enumerate(bf_taps):
                        nc.tensor.matmul(
                            ps_x0[:, c0:c1], wbs[:, 1, cc, ki, oc],
                            tracks[cc][0][:, 1, kb + c0 : kb + c1],
                            start=(cc == 0 and ki == 0),
                            stop=(not NQ and cc == CCH - 1 and ki == NB - 1),
                        )
                        nc.tensor.matmul(
                            ps_dx[:, c0:c1], wbs[:, 1, cc, ki, oc],
                            tracks[cc][0][:, 2, kb + c0 : kb + c1],
                            start=(cc == 0 and ki == 0),
                            stop=False,
                        )
                # dx part 2: w3 on track1
                bf_mms(ps_dx, 2, 1, False, not NQ)
                if NQ:
                    q_mms(ps_x, 0, 0, True)
                    q_mms(ps_x0, 1, 1, True)
                    q_mms(ps_dx, 1, 2, False)
                    q_mms(ps_dx, 2, 1, True)
                return ps_x, ps_x0, ps_dx

            # ---- program start: first slab, first weight chunk, second
            # slab, then the remaining weight chunks in consumption order ----
            def dma_wchunk(ti, cc, oc_list=None, with_q=True):
                base = (ti * CCH + cc) * NB * OCH * P
                src = wb[:, base : base + NB * OCH * P].rearrange(
                    "p (k o q) -> p k o q", k=NB, o=OCH)
                if oc_list is None:
                    nc.sync.dma_start(wbs[:, ti, cc], src)
                else:
                    # finer pieces so the first matmul's weights land sooner
                    for oc in oc_list:
                        nc.sync.dma_start(wbs[:, ti, cc, :, oc], src[:, :, oc])
                if NQ and with_q:
                    nc.sync.dma_start(
                        wqs[:, ti, cc],
                        wq[:, (ti * CCH + cc) * NQ * OCH * 2 * P :
                              (ti * CCH + cc + 1) * NQ * OCH * 2 * P]
                        .rearrange("p (k o i q) -> p k o i q",
                                   k=NQ, o=OCH, i=2),
                    )

            slab0 = dma_slabs(
                0,
                interleave_dma=lambda ti, cc: dma_wchunk(
                    0, 0, oc_list=(0,), with_q=False),
            )
            dma_wchunk(0, 0, oc_list=(1,))
            for ti in range(3):
                for cc in range(CCH):
                    if (ti, cc) == (0, 0):
                        continue  # already issued above
                    dma_wchunk(ti, cc)
            nc.sync.dma_start(bss[:], bs[:].rearrange("p (o s) -> p o s", o=OCH))

            for tt in range(NT):
                slab_pair = slab0 if tt == 0 else dma_slabs(tt)
                tracks = prep_tracks(slab_pair)
                for oc in range(OCH):
                    if tt == NT - 1:
                        # split the final tile into sub-groups so the tail
                        # evict+DMA chain after the last matmul is short;
                        # the very last sub-group is only 128 columns
                        splits = ((0, 256), (256, 512)) if oc < OCH - 1 else (
                            (0, 256), (256, 448), (448, 512))
                        for c0, c1 in splits:
                            ps = mm_group(tracks, oc, c0, c1)
                            post(oc, tt * TT, *ps, c0=c0, c1=c1)
                    else:
                        ps_x, ps_x0, ps_dx = mm_group(tracks, oc)
                        post(oc, tt * TT, ps_x, ps_x0, ps_dx)

    ndedup = _dedupe_ldweights(nc)
    if ndedup:
        import logging
        logging.getLogger(__name__).info("deduped %d ldweights", ndedup)
    _split_excess_waits(nc)
    return nc


_CACHE: dict = {}


def _prep_weights(weight, w0, w, alpha, fp8_taps=FP8_TAPS):
    """(O, C, K) fp32 -> bf16 lhsT chunks + e4m3 (hi, lo) pair chunks."""
    import ml_dtypes

    E4 = ml_dtypes.float8_e4m3
    BF = ml_dtypes.bfloat16
    fp8_taps = tuple(fp8_taps)
    bf_taps = tuple(k for k in range(K) if k not in fp8_taps)
    s = np.sqrt(np.abs(np.asarray(alpha, np.float32)))  # (1,1,K)
    inv_sqrt_c = np.float32(1.0 / math.sqrt(C))
    wb_parts = []
    wq_parts = []
    for wt in (weight, w0, w):
        wt = np.asarray(wt, np.float32) * s * inv_sqrt_c  # (O, C, K)
        # lhsT layout: [c_lo, cc, k, oc, o_lo]
        lt = wt.reshape(OCH, P, CCH, P, K).transpose(3, 2, 4, 0, 1)
        # bf16 taps
        wb_parts.append(np.ascontiguousarray(lt[:, :, bf_taps]).astype(BF))
        if fp8_taps:
            q = lt[:, :, fp8_taps]  # (P, CCH, NQ, OCH, P)
            hi = q.astype(E4).astype(np.float32)
            lo = (q - hi).astype(E4)
            pair = np.stack([hi.astype(E4), lo], axis=-2)  # (..., 2, P)
            wq_parts.append(np.ascontiguousarray(pair))
    wb_np = np.concatenate([p.reshape(P, -1) for p in wb_parts], axis=1)
    wq_np = (np.concatenate([p.reshape(P, -1) for p in wq_parts], axis=1)
             if fp8_taps else None)
    return wb_np, wq_np


def kernel(x, weight, w0, w, alpha, bias, b0, b, beta):
    import ml_dtypes
    from concourse.bass_utils import run_bass_kernel_spmd

    x = np.asarray(x, np.float32)
    wb_np, wq_np = _prep_weights(weight, w0, w, alpha)
    sb = np.float32(math.sqrt(abs(float(np.asarray(beta)))))
    biases = np.stack(
        [np.asarray(bias, np.float32) * sb,
         np.asarray(b0, np.float32) * sb,
         np.asarray(b, np.float32) * sb],
        axis=-1,
    )  # (O, 3) in track order [x, x0, dx]
    bs_np = np.ascontiguousarray(biases.reshape(OCH, P, 3).transpose(1, 0, 2)).reshape(
        P, OCH * 3
    )

    if "nc" not in _CACHE:
        _CACHE["nc"] = _build_nc()
    nc = _CACHE["nc"]

    xh = x.reshape(B, C, T * 3).astype(ml_dtypes.bfloat16)
    in_maps = []
    for c in range(NCORES):
        im = {
            "xd": np.ascontiguousarray(xh[c]),
            "wb": wb_np,
            "bs": bs_np,
        }
        if wq_np is not None:
            im["wq"] = wq_np
        in_maps.append(im)
    res = run_bass_kernel_spmd(nc, in_maps, list(range(NCORES)))
    out = np.empty((B, C, T, 3), np.float32)
    for c in range(NCORES):
        out[c] = res.results[c]["yd"].reshape(C, T, 3)
    return out


# revision 21
# speedup vs baseline: 1.9893x; 1.0007x over previous
"""TRN2 Bass kernel for the NTK-track Conv1d problem.

Reference computation (per batch element b, all fp32):
    xv = relu(x[...,0]); x0 = relu(x[...,1]); dx = x[...,2] * (x[...,1] >= 0)
    s = sqrt(|alpha|)  (per-tap scale, K=9)
    x_out  = conv1d(xv, weight*s)/sqrt(C) + bias*sqrt(|beta|)
    x0_out = conv1d(x0, w0*s)/sqrt(C)     + b0*sqrt(|beta|)
    dx_out = (conv1d(dx, w0*s) + conv1d(x0, w*s))/sqrt(C) + b*sqrt(|beta|)
    out = stack([x_out, x0_out, dx_out], -1)

Shapes: x (8, 256, 8192, 3); weight/w0/w (256, 256, 9); pad=4 (same conv).

Strategy: data-parallel over batch (8 cores, 1 batch element each).
Per core the four conv streams (xv*W1, x0*W2, dx*W2, x0*W3) are shifted
128x128 matmuls accumulated in PSUM (contraction over C chunks and taps).
Mixed precision per tap:
  - taps NOT in FP8_TAPS run in bf16 (1 col/cycle, ~2e-3 global err),
  - taps in FP8_TAPS run as e4m3 DoubleRow matmuls (2 cols/cycle): the
    stationary operand packs host-prepared (w_hi, w_lo) e4m3 pairs so the
    weight is ~7-mantissa-bit accurate, the moving operand is a single
    e4m3 copy of the track broadcast across the pair dim; the only extra
    error is the track quantization on those taps (~6e-3/tap global).
The 1/sqrt(C) and sqrt(|alpha|) factors are folded into the weights on the
host; the sqrt(|beta|)-scaled biases are added during PSUM->SBUF eviction.
Input x is pre-converted to bf16 on the host (halves the slab DMA).

DMA order is tuned so compute starts early: the first time-tile's input
slabs are fetched first, then weight chunks in exact consumption order.
"""

import math

import numpy as np

B, C, O, T, K = 8, 256, 256, 8192, 9
PAD = 4
P = 128  # partitions
TT = 512  # time-tile (matmul free dim)
NT = T // TT  # 16 time tiles
CCH = C // P  # 2 contraction chunks
OCH = O // P  # 2 output-partition chunks
HALO = TT + 2 * PAD  # 520 input columns per tile
NCORES = 8
FP8_TAPS = (0, 8)  # taps computed in e4m3 DoubleRow (2x PE rate)


def _split_excess_waits(nc) -> int:
    """Move excess per-instruction semaphore waits onto standalone
    EventSemaphore carrier instructions.

    The walrus build in this environment rejects any instruction carrying
    more than ONE sync wait at codegen ("Too many sync wait commands");
    Tile's sem assignment freely emits several. Walk the finished BIR and
    hoist overflow waits onto fresh same-engine EventSemaphore instructions
    placed immediately before the over-budget instruction.
    """
    import concourse.mybir as mybir

    n_carriers = 0
    for f in nc.m.functions:
        for blk in f.blocks:
            insts = list(blk.instructions)
            new_insts = []
            dirty = False
            for inst in insts:
                si = inst.sync_info
                waits = list(si.on_wait) if si is not None and si.on_wait else []
                if len(waits) > 1:
                    overflow, keep = waits[:-1], waits[-1:]
                    for w in overflow:
                        ev = mybir.InstEventSemaphore(
                            name=f"{inst.name}_waitc{n_carriers}",
                            engine=inst.engine,
                        )
                        ev.sync_info = mybir.SyncInfo(on_wait=[w], on_update=[])
                        nc.register_instruction(ev, overwrite=True)
                        new_insts.append(ev)
                        n_carriers += 1
                    upd = list(si.on_update) if si.on_update else []
                    inst.sync_info = mybir.SyncInfo(on_wait=keep, on_update=upd)
                    dirty = True
                new_insts.append(inst)
            if dirty:
                blk.instructions = new_insts
    return n_carriers


def _dedupe_ldweights(nc) -> int:
    """Drop an InstLdweights whose weights AP matches the previous kept
    InstLdweights on the same stream with only Matmult / EventSemaphore
    instructions in between (the PE array still holds those weights).
    Waits from a dropped LDW migrate to the next kept PE instruction.
    Must run BEFORE _split_excess_waits so merged waits get re-split."""
    import concourse.mybir as mybir

    removed = 0
    for f in nc.m.functions:
        for blk in f.blocks:
            insts = list(blk.instructions)
            new_insts = []
            last_ld_key = None
            pend_waits = []
            for inst in insts:
                op = inst.opcode
                if op == "Ldweights":
                    key = str(inst.ins[0])
                    if key == last_ld_key:
                        si = inst.sync_info
                        if si is not None and si.on_wait:
                            pend_waits.extend(list(si.on_wait))
                        if si is not None and si.on_update:
                            # don't drop an LDW other procs wait on
                            new_insts.append(inst)
                            continue
                        removed += 1
                        continue
                    last_ld_key = key
                elif op in ("Matmult", "EventSemaphore"):
                    pass  # doesn't clobber the loaded weights
                else:
                    last_ld_key = None
                if pend_waits and inst.engine == mybir.EngineType.PE:
                    si = inst.sync_info
                    w = list(si.on_wait) if si is not None and si.on_wait else []
                    u = list(si.on_update) if si is not None and si.on_update else []
                    inst.sync_info = mybir.SyncInfo(on_wait=pend_waits + w, on_update=u)
                    pend_waits = []
                new_insts.append(inst)
            if removed:
                assert not pend_waits
                blk.instructions = new_insts
    return removed


def _build_nc(fp8_taps: tuple = FP8_TAPS, warmup: int = 8):
    import concourse.bass as bass
    import concourse.mybir as mybir
    from concourse.tile import TileContext

    f32 = mybir.dt.float32
    bf16 = mybir.dt.bfloat16
    f8 = mybir.dt.float8e4
    AF = mybir.ActivationFunctionType
    OP = mybir.AluOpType
    PM = mybir.MatmulPerfMode.DoubleRow

    fp8_taps = tuple(fp8_taps)
    bf_taps = tuple(k for k in range(K) if k not in fp8_taps)
    NB = len(bf_taps)
    NQ = len(fp8_taps)
    NH = TT // 256  # DoubleRow half-tiles per time tile

    nc = bass.Bass()
    xd = nc.declare_dram_parameter("xd", [C, T * 3], bf16, isOutput=False)
    # bf16 weights: [c_lo, tensor, cc, kb, oc, o_lo] flattened after c_lo
    wb = nc.declare_dram_parameter("wb", [P, 3 * CCH * NB * OCH * P], bf16,
                                   isOutput=False)
    # fp8 (hi, lo) pairs: [c_lo, tensor, cc, kq, oc, pair, o_lo]
    if NQ:
        wq = nc.declare_dram_parameter("wq", [P, 3 * CCH * NQ * OCH * 2 * P], f8,
                                       isOutput=False)
    bs = nc.declare_dram_parameter("bs", [P, OCH * 3], f32, isOutput=False)
    yd = nc.declare_dram_parameter("yd", [C, T * 3], f32, isOutput=True)

    with TileContext(nc) as tc:
        with (
            tc.tile_pool(name="wpool", bufs=1) as wpool,
            tc.tile_pool(name="slabs", bufs=4) as slabs,
            tc.tile_pool(name="trks", bufs=4) as trks,
            tc.tile_pool(name="opool", bufs=4) as opool,
            tc.tile_pool(name="psum", bufs=2, space="PSUM") as psp,
            tc.tile_pool(name="psumw", bufs=1, space="PSUM") as psw,
        ):
            # Persistent weights / biases
            wbs = wpool.tile([P, 3, CCH, NB, OCH, P], bf16)
            if NQ:
                wqs = wpool.tile([P, 3, CCH, NQ, OCH, 2, P], f8)
            bss = wpool.tile([P, OCH, 3], f32)

            if warmup:
                # Dummy matmuls on zeroed scratch bridge the PE through the
                # initial input/weight DMA wait: the p-state ramp (full
                # speed only after 3us of continuous PE busy) completes
                # before the first real matmul instead of slowing it down.
                scr = wpool.tile([P, TT], bf16)
                psd0 = psw.tile([P, TT], f32)
                nc.vector.memset(scr[:], 0.0)
                for _ in range(warmup):
                    nc.tensor.matmul(psd0[:], scr[:, 0:P], scr[:],
                                     start=True, stop=True)

            def dma_slabs(tt, interleave_dma=None):
                t0 = tt * TT
                out = []
                for cc in range(CCH):
                    if interleave_dma is not None and cc:
                        # keep the DMA queue in consumption order: the first
                        # weight chunk lands right after the first slab
                        interleave_dma(0, cc - 1)
                    slab = slabs.tile([P, HALO * 3], bf16, tag="slab")
                    lo = 3 * (t0 - PAD)
                    hi = 3 * (t0 + TT + PAD)
                    zlo = max(0, -lo)      # zero-pad columns at the left edge
                    zhi = max(0, hi - 3 * T)  # and at the right edge
                    if zlo:
                        nc.vector.memset(slab[:, :zlo], 0.0)
                    if zhi:
                        nc.vector.memset(slab[:, HALO * 3 - zhi:], 0.0)
                    nc.sync.dma_start(
                        slab[:, zlo : HALO * 3 - zhi],
                        xd[cc * P : (cc + 1) * P, lo + zlo : hi - zhi],
                    )
                    out.append(slab)
                return out

            def prep_tracks(slab_pair):
                # phase 1: bf16 tracks for all cc chunks (these gate the
                # first matmuls of the group); phase 2: fp8 copies (used by
                # the DoubleRow passes at the end of each group)
                tracks = []
                for cc in range(CCH):
                    slab = slab_pair[cc]
                    sv = slab[:].rearrange("p (t s) -> p t s", s=3)
                    trk = trks.tile([P, 3, HALO], bf16, tag="trk")
                    # xv = relu(track0); x0 = relu(track1)  (ACT engine)
                    nc.scalar.activation(trk[:, 0], sv[:, :, 0], AF.Relu)
                    nc.scalar.activation(trk[:, 1], sv[:, :, 1], AF.Relu)
                    # dx = track2 * (track1 >= 0)  (DVE engine)
                    msk = trks.tile([P, HALO], bf16, tag="msk")
                    nc.vector.tensor_scalar(msk[:], sv[:, :, 1], 0.0, None, OP.is_ge)
                    nc.vector.tensor_tensor(trk[:, 2], msk[:], sv[:, :, 2], OP.mult)
                    tracks.append((trk, None))
                if NQ:
                    for cc in range(CCH):
                        slab = slab_pair[cc]
                        sv = slab[:].rearrange("p (t s) -> p t s", s=3)
                        trk = tracks[cc][0]
                        x8 = trks.tile([P, 3, HALO], f8, tag="x8")
                        # fp8 copies of the tracks for the DoubleRow taps
                        nc.scalar.activation(x8[:, 0], sv[:, :, 0], AF.Relu)
                        nc.scalar.activation(x8[:, 1], sv[:, :, 1], AF.Relu)
                        nc.vector.tensor_copy(x8[:, 2], trk[:, 2])
                        tracks[cc] = (trk, x8)
                return tracks

            def post(oc, t0, ps_x, ps_x0, ps_dx, c0=0, c1=TT):
                w = c1 - c0
                ot = opool.tile([P, w, 3], f32, tag=f"ot{w}")
                # x-track eviction on ACT (bias via activation), rest on DVE
                nc.scalar.activation(ot[:, :, 0], ps_x[:, c0:c1], AF.Identity,
                                     bias=bss[:, oc, 0:1])
                nc.vector.tensor_scalar_add(ot[:, :, 1], ps_x0[:, c0:c1], bss[:, oc, 1:2])
                nc.vector.tensor_scalar_add(ot[:, :, 2], ps_dx[:, c0:c1], bss[:, oc, 2:3])
                nc.sync.dma_start(
                    yd[oc * P : (oc + 1) * P, 3 * (t0 + c0) : 3 * (t0 + c1)],
                    ot[:].rearrange("p t s -> p (t s)"),
                )

            def mm_group(tracks, oc, c0=0, c1=TT):
                """One accumulation group: all taps of the 4 conv streams for
                one (time-tile, oc), over output columns [c0, c1). Streams:
                t0=w1*trk0, t1=w2*trk1, t2=w2*trk2 + w3*trk1 (dx)."""
                w = c1 - c0
                ps_x = psp.tile([P, TT], f32, tag="psx")
                ps_x0 = psp.tile([P, TT], f32, tag="psx0")
                ps_dx = psp.tile([P, TT], f32, tag="psdx")

                def bf_mms(ps, ti, tr, first, last):
                    # bf16 taps of conv stream (weight tensor ti, track tr)
                    for ci, cc in enumerate(range(CCH)):
                        for ki, kb in enumerate(bf_taps):
                            nc.tensor.matmul(
                                ps[:, c0:c1], wbs[:, ti, cc, ki, oc],
                                tracks[cc][0][:, tr, kb + c0 : kb + c1],
                                start=(first and ci == 0 and ki == 0),
                                stop=(last and ci == CCH - 1 and ki == NB - 1),
                            )

                def q_mms(ps, ti, tr, last):
                    # fp8 DoubleRow taps (half tiles of <=256 columns)
                    for ci, cc in enumerate(range(CCH)):
                        x8 = tracks[cc][1]
                        for ki, kq in enumerate(fp8_taps):
                            for h0 in range(c0, c1, 256):
                                hw = min(256, c1 - h0)
                                rhs = x8[:, tr : tr + 1, kq + h0 : kq + h0 + hw]
                                nc.tensor.matmul(
                                    ps[:, h0 : h0 + hw],
                                    wqs[:, ti, cc, ki, oc],
                                    rhs.broadcast_to([P, 2, hw]),
                                    start=False,
                                    stop=(last and ci == CCH - 1
                                          and ki == NQ - 1),
                                    perf_mode=PM,
                                )

                # bf16 passes first (their tracks/weights are ready
                # earliest at kernel start), fp8 DoubleRow passes last.
                # x stream: w1 on track0
                bf_mms(ps_x, 0, 0, True, not NQ)
                # x0 stream (w2*trk1) and dx part 1 (w2*trk2): adjacent
                # matmuls share the w2 stationary tile (ldweights dedupe)
                for cc in range(CCH):
                    for ki, kb in enumerate(bf_taps):
                        nc.tensor.matmul(
                            ps_x0[:, c0:c1], wbs[:, 1, cc, ki, oc],
                            tracks[cc][0][:, 1, kb + c0 : kb + c1],
                            start=(cc == 0 and ki == 0),
                            stop=(not NQ and cc == CCH - 1 and ki == NB - 1),
                        )
                        nc.tensor.matmul(
                            ps_dx[:, c0:c1], wbs[:, 1, cc, ki, oc],
                            tracks[cc][0][:, 2, kb + c0 : kb + c1],
                            start=(cc == 0 and ki == 0),
                            stop=False,
                        )
                # dx part 2: w3 on track1
                bf_mms(ps_dx, 2, 1, False, not NQ)
                if NQ:
                    q_mms(ps_x, 0, 0, True)
                    q_mms(ps_x0, 1, 1, True)
                    q_mms(ps_dx, 1, 2, False)
                    q_mms(ps_dx, 2, 1, True)
                return ps_x, ps_x0, ps_dx

            # ---- program start: first slab, first weight chunk, second
            # slab, then the remaining weight chunks in consumption order ----
            def dma_wchunk_bf(ti, cc, oc):
                base = (ti * CCH + cc) * NB * OCH * P
                src = wb[:, base : base + NB * OCH * P].rearrange(
                    "p (k o q) -> p k o q", k=NB, o=OCH)
                nc.sync.dma_start(wbs[:, ti, cc, :, oc], src[:, :, oc])

            def dma_wchunk_q(ti, cc):
                nc.sync.dma_start(
                    wqs[:, ti, cc],
                    wq[:, (ti * CCH + cc) * NQ * OCH * 2 * P :
                          (ti * CCH + cc + 1) * NQ * OCH * 2 * P]
                    .rearrange("p (k o i q) -> p k o i q",
                               k=NQ, o=OCH, i=2),
                )

            # weight chunks are issued in exact tt0 consumption order (oc0's
            # bf16 slices, fp8 pairs, then oc1's), interleaved with the tt0
            # slab fetches, so the PE never waits on the serial DMA queue
            slab0 = dma_slabs(
                0, interleave_dma=lambda ti, cc: dma_wchunk_bf(0, 0, 0))
            dma_wchunk_bf(0, 1, 0)
            for ti in (1, 2):
                for cc in range(CCH):
                    dma_wchunk_bf(ti, cc, 0)
            if NQ:
                for ti in range(3):
                    for cc in range(CCH):
                        dma_wchunk_q(ti, cc)
            for ti in range(3):
                for cc in range(CCH):
                    dma_wchunk_bf(ti, cc, 1)
            nc.sync.dma_start(bss[:], bs[:].rearrange("p (o s) -> p o s", o=OCH))

            for tt in range(NT):
                slab_pair = slab0 if tt == 0 else dma_slabs(tt)
                tracks = prep_tracks(slab_pair)
                for oc in range(OCH):
                    if tt == NT - 1:
                        # split the final tile into sub-groups so the tail
                        # evict+DMA chain after the last matmul is short;
                        # the very last sub-group is only 128 columns
                        splits = ((0, 256), (256, 512)) if oc < OCH - 1 else (
                            (0, 256), (256, 448), (448, 512))
                        for c0, c1 in splits:
                            ps = mm_group(tracks, oc, c0, c1)
                            post(oc, tt * TT, *ps, c0=c0, c1=c1)
                    else:
                        ps_x, ps_x0, ps_dx = mm_group(tracks, oc)
                        post(oc, tt * TT, ps_x, ps_x0, ps_dx)

    ndedup = _dedupe_ldweights(nc)
    if ndedup:
        import logging
        logging.getLogger(__name__).info("deduped %d ldweights", ndedup)
    _split_excess_waits(nc)
    return nc


_CACHE: dict = {}


def _prep_weights(weight, w0, w, alpha, fp8_taps=FP8_TAPS):
    """(O, C, K) fp32 -> bf16 lhsT chunks + e4m3 (hi, lo) pair chunks."""
    import ml_dtypes

    E4 = ml_dtypes.float8_e4m3
    BF = ml_dtypes.bfloat16
    fp8_taps = tuple(fp8_taps)
    bf_taps = tuple(k for k in range(K) if k not in fp8_taps)
    s = np.sqrt(np.abs(np.asarray(alpha, np.float32)))  # (1,1,K)
    inv_sqrt_c = np.float32(1.0 / math.sqrt(C))
    wb_parts = []
    wq_parts = []
    for wt in (weight, w0, w):
        wt = np.asarray(wt, np.float32) * s * inv_sqrt_c  # (O, C, K)
        # lhsT layout: [c_lo, cc, k, oc, o_lo]
        lt = wt.reshape(OCH, P, CCH, P, K).transpose(3, 2, 4, 0, 1)
        # bf16 taps
        wb_parts.append(np.ascontiguousarray(lt[:, :, bf_taps]).astype(BF))
        if fp8_taps:
            q = lt[:, :, fp8_taps]  # (P, CCH, NQ, OCH, P)
            hi = q.astype(E4).astype(np.float32)
            lo = (q - hi).astype(E4)
            pair = np.stack([hi.astype(E4), lo], axis=-2)  # (..., 2, P)
            wq_parts.append(np.ascontiguousarray(pair))
    wb_np = np.concatenate([p.reshape(P, -1) for p in wb_parts], axis=1)
    wq_np = (np.concatenate([p.reshape(P, -1) for p in wq_parts], axis=1)
             if fp8_taps else None)
    return wb_np, wq_np


def kernel(x, weight, w0, w, alpha, bias, b0, b, beta):
    import ml_dtypes
    from concourse.bass_utils import run_bass_kernel_spmd

    x = np.asarray(x, np.float32)
    wb_np, wq_np = _prep_weights(weight, w0, w, alpha)
    sb = np.float32(math.sqrt(abs(float(np.asarray(beta)))))
    biases = np.stack(
        [np.asarray(bias, np.float32) * sb,
         np.asarray(b0, np.float32) * sb,
         np.asarray(b, np.float32) * sb],
        axis=-1,
    )  # (O, 3) in track order [x, x0, dx]
    bs_np = np.ascontiguousarray(biases.reshape(OCH, P, 3).transpose(1, 0, 2)).reshape(
        P, OCH * 3
    )

    if "nc" not in _CACHE:
        _CACHE["nc"] = _build_nc()
    nc = _CACHE["nc"]

    xh = x.reshape(B, C, T * 3).astype(ml_dtypes.bfloat16)
    in_maps = []
    for c in range(NCORES):
        im = {
            "xd": np.ascontiguousarray(xh[c]),
            "wb": wb_np,
            "bs": bs_np,
        }
        if wq_np is not None:
            im["wq"] = wq_np
        in_maps.append(im)
    res = run_bass_kernel_spmd(nc, in_maps, list(range(NCORES)))
    out = np.empty((B, C, T, 3), np.float32)
    for c in range(NCORES):
        out[c] = res.results[c]["yd"].reshape(C, T, 3)
    return out
